# revision 47
# baseline (speedup 1.0000x reference)
"""Causal multi-head attention (B=1, S=4096, D=1024, H=16, HD=64) on 8 TRN2
NeuronCores.

Sharding: tensor-parallel over heads. Core c owns heads [2c, 2c+1]: Wq/Wk/Wv
column slices (128 cols) and Wo row slice (128 rows). Each core computes its
partial output projection over the full sequence; the host sums the 8 partials
and adds bo (the row-parallel all-reduce done at gather time).

Device kernel layout (per core, all matmuls float32r = full-rate PE):
  - x is fed pre-transposed (xT [D, S]) so the QKV projections need no
    on-chip transpose: Q^T/K^T/V^T [c=128(2 heads x 64), s] = W.T @ xT.
  - scores are computed TRANSPOSED: scT[sk, sq] = K_h @ Q_h^T per 128-row
    sk-tile, which makes exp(scores) directly usable as the stationary
    operand of attn@V -- no P transposes. Softmax denominators come for free
    from a ones-column appended to V (row 64 of the attn@V accumulator);
    the 1/sqrt(HD) scale is folded into Wq/bq on the host; max-subtraction
    is skipped (scores ~ N(0,1), exp is safe in fp32).
  - V^T -> V via PE transposes of [128,128] blocks.
  - causal masking: post-exp zeroing (gpsimd memset) + upper-tri mask
    multiply on diagonal 128x128 blocks (gpsimd).
  - rescale by the denominator reciprocal (fast DVE recip + gpsimd
    partition-broadcast), folded in before the output projection.
"""
import sys

sys.path.insert(0, "/opt/trn_rl_repo")

import numpy as np

import concourse.bacc as bacc
import concourse.mybir as mybir
import concourse.tile as tile
from concourse.bass_utils import run_bass_kernel_spmd
from concourse.dve_ops import RECIPROCAL_APPROX_FAST, RECIP_APPROX_FAST_CONSTS

F32 = mybir.dt.float32
F32R = mybir.dt.float32r
BF16 = mybir.dt.bfloat16
USE_BF16 = True
CDT = BF16 if USE_BF16 else F32R  # matmul-operand compute dtype
EXP = mybir.ActivationFunctionType.Exp

S, D, H, HD = 4096, 1024, 16, 64
NCORES = 8
CPC = D // NCORES  # 128 head-dim columns per core (2 heads)
NS = S // 512      # 8 chunks of 512 along the sequence
ND = D // 128      # 8 contraction chunks for the projections


def _build_nc(dbg=False):
    nc = bacc.Bacc("TRN2", target_bir_lowering=False, debug=False,
                   num_devices=NCORES)
    xT = nc.dram_tensor("xT", [D, S], CDT, kind="ExternalInput").ap()
    w3 = nc.dram_tensor("w3", [128, 3, ND, 128], CDT, kind="ExternalInput").ap()
    wo = nc.dram_tensor("wo", [128, D], CDT, kind="ExternalInput").ap()
    bq = nc.dram_tensor("bq", [CPC, 1], F32, kind="ExternalInput").ap()
    bk = nc.dram_tensor("bk", [CPC, 1], F32, kind="ExternalInput").ap()
    bv = nc.dram_tensor("bv", [CPC, 1], F32, kind="ExternalInput").ap()
    tri = nc.dram_tensor("tri", [128, 4, 512], CDT, kind="ExternalInput").ap()
    ident = nc.dram_tensor("ident", [128, 128], F32, kind="ExternalInput").ap()
    out = nc.dram_tensor("out", [S, D], F32, kind="ExternalOutput").ap()
    dbg_t = {}
    if dbg:
        for name, shape in (("d_QT", [128, S]), ("d_KT", [128, S]),
                            ("d_VP", [128, 2, 32, 65]), ("d_OT0", [64, S]),
                            ("d_OT1", [64, S]), ("d_pt", [128, 8, 1024]),
                            ("d_ot0", [65, 512]), ("d_ot1", [65, 512])):
            dbg_t[name] = nc.dram_tensor(name, shape, F32R,
                                         kind="ExternalOutput").ap()
        for name, shape in (("d_rec", [2, 512]), ("d_bc", [64, 512])):
            dbg_t[name] = nc.dram_tensor(name, shape, F32,
                                         kind="ExternalOutput").ap()

    with tile.TileContext(nc) as tc:
        _emit(nc, tc, xT, w3, wo, bq, bk, bv, tri, ident, out, dbg_t)
    nc.compile()
    return nc


def _emit(nc, tc, xT, w3, wo, bq, bk, bv, tri, ident, out, dbg_t=None):
    from contextlib import ExitStack
    ctx = ExitStack()
    with ctx:
        consts = ctx.enter_context(tc.tile_pool(name="consts", bufs=1))
        persist = ctx.enter_context(tc.tile_pool(name="persist", bufs=1))
        xt_pool = ctx.enter_context(tc.tile_pool(name="xt", bufs=24))
        vt_pool = ctx.enter_context(tc.tile_pool(name="vt", bufs=2))
        pt_pool = ctx.enter_context(tc.tile_pool(name="pt", bufs=6))
        rec_pool = ctx.enter_context(tc.tile_pool(name="rec", bufs=2))
        bc_pool = ctx.enter_context(tc.tile_pool(name="bc", bufs=2))
        ost_pool = ctx.enter_context(tc.tile_pool(name="ost", bufs=4))
        ps_mm = ctx.enter_context(tc.tile_pool(name="psmm", bufs=2, space="PSUM"))
        ps_sc = ctx.enter_context(tc.tile_pool(name="pssc", bufs=2, space="PSUM"))
        ps_ot = ctx.enter_context(tc.tile_pool(name="psot", bufs=2, space="PSUM"))

        # ---- constants (wq slice first so the first matmul starts early;
        # the rest after the first chunk's xt loads) ----------------------
        w_sb = consts.tile([128, 3, ND, 128], CDT)
        nc.sync.dma_start(out=w_sb[:, 0], in_=w3[:, 0])
        bq_sb = consts.tile([CPC, 1], F32)
        bk_sb = consts.tile([CPC, 1], F32)
        bv_sb = consts.tile([CPC, 1], F32)
        nc.sync.dma_start(out=bq_sb, in_=bq)
        nc.sync.dma_start(out=bk_sb, in_=bk)
        nc.sync.dma_start(out=bv_sb, in_=bv)
        id_sb = consts.tile([128, 128], F32)
        tri_sb = consts.tile([128, 4, 512], CDT)
        wo_sb = consts.tile([128, D], CDT)

        def emit_late_consts():
            nc.sync.dma_start(out=w_sb[:, 1], in_=w3[:, 1])
            nc.sync.dma_start(out=w_sb[:, 2], in_=w3[:, 2])
            nc.sync.dma_start(out=tri_sb, in_=tri)
            nc.sync.dma_start(out=id_sb, in_=ident)
            nc.sync.dma_start(out=wo_sb, in_=wo)

        # ---- persistent activations -----------------------------------
        QT = persist.tile([128, S], CDT)      # [c(2 heads x 64), s]
        KT = persist.tile([128, S], CDT)
        VP = persist.tile([128, 2, 32, 65], CDT)  # V natural + ones col, per (h, sk-tile)
        OT = persist.tile([128, S], CDT)      # rescaled attn out^T, both heads

        def emit_proj(c):
            # output projection for s-chunk c (deferred one chunk so the
            # rescale latency chain never stalls the PE queue)
            for t in range(4):
                ss = slice(128 * (4 * c + t), 128 * (4 * c + t + 1))
                for n in range(2):
                    nn = slice(512 * n, 512 * (n + 1))
                    pr_ps = ps_mm.tile([128, 512], F32, tag="mm")
                    nc.tensor.matmul(pr_ps, OT[:, ss], wo_sb[:, nn],
                                     start=True, stop=True)
                    o_sb = ost_pool.tile([128, 512], F32, tag="ost")
                    nc.vector.tensor_copy(o_sb, pr_ps)
                    nc.gpsimd.dma_start(out=out[ss, nn], in_=o_sb)

        for c in range(NS):
            cc = slice(512 * c, 512 * (c + 1))

            # ---- phase A: QKV projections for s-chunk c ---------------
            xts = []
            for k in range(ND):
                xt = xt_pool.tile([128, 512], CDT, tag="xt")
                nc.sync.dma_start(out=xt, in_=xT[128 * k:128 * (k + 1), cc])
                xts.append(xt)
            if c == 0:
                emit_late_consts()
            vt_sb = vt_pool.tile([128, 512], F32, tag="vt")
            for i, (dst, b_sb) in enumerate(((QT[:, cc], bq_sb),
                                             (KT[:, cc], bk_sb),
                                             (vt_sb, bv_sb))):
                p_ps = ps_mm.tile([128, 512], F32, tag="mm")
                for k in range(ND):
                    nc.tensor.matmul(p_ps, w_sb[:, i, k], xts[k],
                                     start=(k == 0), stop=(k == ND - 1))
                nc.vector.tensor_scalar_add(dst, p_ps, b_sb)
            # V^T [c, s] -> V natural [s, c] per 128-block, + ones column
            for t in range(4):
                j = 4 * c + t
                tr_ps = ps_mm.tile([128, 128], F32, tag="mm")
                nc.tensor.transpose(tr_ps, vt_sb[:, 128 * t:128 * (t + 1)], id_sb)
                nc.vector.tensor_copy(VP[:, 0, j, 0:64], tr_ps[:, 0:64])
                nc.vector.tensor_copy(VP[:, 1, j, 0:64], tr_ps[:, 64:128])
                # ones column (Memset has no f32r encoding: use in*0+1)
                for h in (0, 1):
                    nc.vector.tensor_scalar(VP[:, h, j, 64:65],
                                            tri_sb[:, 0, 0:1],
                                            0.0, 1.0, mybir.AluOpType.mult,
                                            mybir.AluOpType.add)

            if c > 0:
                emit_proj(c - 1)

            # ---- phase B: attention for sq chunk c --------------------
            njt = 4 * (c + 1)  # causal sk tiles
            ot0 = ps_ot.tile([65, 512], F32, tag="ot")
            ot1 = ps_ot.tile([65, 512], F32, tag="ot")

            def emit_av(j, pt):
                st, sp = (j == 0), (j == njt - 1)
                nc.tensor.matmul(ot0, VP[:, 0, j], pt[:, 0:512], start=st, stop=sp)
                nc.tensor.matmul(ot1, VP[:, 1, j], pt[:, 512:1024], start=st, stop=sp)

            # software pipeline: attn@V for iteration j-1 is emitted after
            # scores+exp of iteration j, so the in-order PE queue never
            # waits on the ACT engine's exp.
            pending = None
            for j in range(njt):
                jj = slice(128 * j, 128 * (j + 1))
                sc_ps = ps_sc.tile([128, 1024], F32, tag="sc")
                nc.tensor.matmul(sc_ps[:, 0:512], KT[0:64, jj], QT[0:64, cc],
                                 start=True, stop=True)
                nc.tensor.matmul(sc_ps[:, 512:1024], KT[64:128, jj], QT[64:128, cc],
                                 start=True, stop=True)
                pt = pt_pool.tile([128, 1024], CDT, tag="pt")
                nc.scalar.activation(out=pt, in_=sc_ps, func=EXP)
                off = 128 * j - 512 * c
                if off >= 0:  # diagonal tile: mask (zeros below 128j, tri on diag)
                    t = j - 4 * c
                    for half in (0, 512):
                        nc.vector.tensor_mul(pt[:, half:half + off + 128],
                                             pt[:, half:half + off + 128],
                                             tri_sb[:, t, 0:off + 128])
                if pending is not None:
                    emit_av(*pending)
                pending = (j, pt)
            emit_av(*pending)

            if dbg_t and c == 1:
                for name, ot in (("d_ot0", ot0), ("d_ot1", ot1)):
                    o_cp = ost_pool.tile([65, 512], F32R, tag="dbg")
                    nc.vector.tensor_copy(o_cp, ot)
                    nc.sync.dma_start(out=dbg_t[name], in_=o_cp)

            # softmax denominators (row 64) -> rescale
            for h, ot in ((0, ot0), (1, ot1)):
                den = rec_pool.tile([1, 512], F32, tag="den")
                nc.vector.tensor_copy(den, ot[64:65, :])
                rec = rec_pool.tile([1, 512], F32, tag="rec")
                nc.vector._custom_dve(RECIPROCAL_APPROX_FAST, out=rec,
                                      in0=den,
                                      s0=RECIP_APPROX_FAST_CONSTS["s0"],
                                      s1=RECIP_APPROX_FAST_CONSTS["s1"],
                                      imm2=RECIP_APPROX_FAST_CONSTS["imm2"])
                bc = bc_pool.tile([64, 512], F32, tag="bc")
                nc.gpsimd.partition_broadcast(bc, rec)
                if dbg_t and c == 1:
                    nc.sync.dma_start(out=dbg_t["d_rec"][h:h + 1], in_=rec)
                    if h == 0:
                        nc.sync.dma_start(out=dbg_t["d_bc"], in_=bc)
                nc.vector.tensor_mul(OT[64 * h:64 * (h + 1), cc], ot[0:64, :], bc)

        emit_proj(NS - 1)

        if dbg_t:
            nc.sync.dma_start(out=dbg_t["d_QT"], in_=QT)
            nc.sync.dma_start(out=dbg_t["d_KT"], in_=KT)
            nc.sync.dma_start(out=dbg_t["d_VP"], in_=VP)
            nc.sync.dma_start(out=dbg_t["d_OT0"], in_=OT[0:64])
            nc.sync.dma_start(out=dbg_t["d_OT1"], in_=OT[64:128])


_NC_CACHE = {}


def _get_nc():
    if "nc" not in _NC_CACHE:
        _NC_CACHE["nc"] = _build_nc()
    return _NC_CACHE["nc"]


def make_in_maps(x, Wq, bq, Wk, bk, Wv, bv, Wo, bo):
    import ml_dtypes
    cdt = ml_dtypes.bfloat16 if USE_BF16 else np.float32
    x = np.asarray(x, np.float32).reshape(S, D)
    xT = np.ascontiguousarray(x.T).astype(cdt)
    scale = 1.0 / np.sqrt(HD)
    # tri[:, t, :]: zeros on cols < 128t, upper-tri on cols [128t, 128t+128),
    # ones beyond (multiplied region is cols [0, 128t+128) of a 512 chunk)
    tri = np.ones((128, 4, 512), np.float32)
    triu = np.triu(np.ones((128, 128), np.float32))
    for t in range(4):
        tri[:, t, :128 * t] = 0.0
        tri[:, t, 128 * t:128 * (t + 1)] = triu
    ident = np.eye(128, dtype=np.float32)
    in_maps = []
    for c in range(NCORES):
        cs = slice(CPC * c, CPC * (c + 1))
        # w3[p, proj, k, c2] = W[128k+p, c2] for the three projections
        w3 = np.stack([np.asarray(Wq)[:, cs] * scale,
                       np.asarray(Wk)[:, cs],
                       np.asarray(Wv)[:, cs]], axis=1)  # [D, 3, 128]
        w3 = np.ascontiguousarray(
            w3.reshape(ND, 128, 3, CPC).transpose(1, 2, 0, 3)).astype(cdt)
        in_maps.append({
            "xT": xT,
            "w3": w3,
            "wo": np.ascontiguousarray(np.asarray(Wo)[cs, :]).astype(cdt),
            "bq": np.ascontiguousarray(np.asarray(bq)[cs] * scale).reshape(CPC, 1),
            "bk": np.ascontiguousarray(np.asarray(bk)[cs]).reshape(CPC, 1),
            "bv": np.ascontiguousarray(np.asarray(bv)[cs]).reshape(CPC, 1),
            "tri": tri.astype(cdt),
            "ident": ident,
        })
    return in_maps


def kernel(x, Wq, bq, Wk, bk, Wv, bv, Wo, bo, _run_kwargs=None):
    nc = _get_nc()
    in_maps = make_in_maps(x, Wq, bq, Wk, bk, Wv, bv, Wo, bo)
    res = run_bass_kernel_spmd(nc, in_maps, list(range(NCORES)),
                               **(_run_kwargs or {}))
    acc = np.zeros((S, D), np.float64)
    for c in range(NCORES):
        acc += res.results[c]["out"]
    full = (acc + np.asarray(bo, np.float64)).astype(np.float32)
    if _run_kwargs is not None:
        _NC_CACHE["last_results"] = res
    return full.reshape(1, S, D)


# revision 49
# speedup vs baseline: 1.0158x; 1.0158x over previous
"""Causal multi-head attention (B=1, S=4096, D=1024, H=16, HD=64) on 8 TRN2
NeuronCores.

Sharding: tensor-parallel over heads. Core c owns heads [2c, 2c+1]: Wq/Wk/Wv
column slices (128 cols) and Wo row slice (128 rows). Each core computes its
partial output projection over the full sequence; the host sums the 8 partials
and adds bo (the row-parallel all-reduce done at gather time).

Device kernel layout (per core; matmul operands in bf16, fp32 PSUM accum,
flip USE_BF16 off for float32r if tighter accuracy is ever needed):
  - x is fed pre-transposed (xT [D, S]) so the QKV projections need no
    on-chip transpose: Q^T/K^T/V^T [c=128(2 heads x 64), s] = W.T @ xT.
  - scores are computed TRANSPOSED: scT[sk, sq] = K_h @ Q_h^T per 128-row
    sk-tile (the two heads' K=64 matmuls run concurrently via PE row
    tiling), which makes exp(scores) directly usable as the stationary
    operand of attn@V -- no P transposes. Softmax denominators come for
    free from a ones-column appended to V (row 64 of the attn@V
    accumulator); the 1/sqrt(HD) scale is folded into Wq/bq on the host;
    max-subtraction is skipped (scores ~ N(0,1), exp is safe in fp32).
  - V^T -> V via PE transposes of [128,128] blocks.
  - causal masking: one DVE multiply per diagonal tile against a
    host-provided mask that is zero below the 128-aligned diagonal band
    and upper-triangular on it.
  - rescale by the denominator reciprocal (fast DVE recip + gpsimd
    partition-broadcast), applied to attn-out^T before the output
    projection (the divide commutes through the linear projection).
  - the attn@V matmuls are software-pipelined one sk-tile behind the
    scores/exp so the in-order PE queue never waits on the ACT engine,
    and each chunk's output projection is deferred one chunk so the
    rescale latency chain is hidden behind the next chunk's QKV matmuls.
"""
import sys

sys.path.insert(0, "/opt/trn_rl_repo")

import numpy as np

import concourse.bacc as bacc
import concourse.mybir as mybir
import concourse.tile as tile
from concourse.bass_utils import run_bass_kernel_spmd
from concourse.dve_ops import RECIPROCAL_APPROX_FAST, RECIP_APPROX_FAST_CONSTS

F32 = mybir.dt.float32
F32R = mybir.dt.float32r
BF16 = mybir.dt.bfloat16
USE_BF16 = True
CDT = BF16 if USE_BF16 else F32R  # matmul-operand compute dtype
EXP = mybir.ActivationFunctionType.Exp

S, D, H, HD = 4096, 1024, 16, 64
NCORES = 8
CPC = D // NCORES  # 128 head-dim columns per core (2 heads)
NS = S // 512      # 8 chunks of 512 along the sequence
ND = D // 128      # 8 contraction chunks for the projections


def _build_nc(dbg=False):
    nc = bacc.Bacc("TRN2", target_bir_lowering=False, debug=False,
                   num_devices=NCORES)
    xT = nc.dram_tensor("xT", [D, S], CDT, kind="ExternalInput").ap()
    w3 = nc.dram_tensor("w3", [128, 3, ND, 128], CDT, kind="ExternalInput").ap()
    wo = nc.dram_tensor("wo", [128, D], CDT, kind="ExternalInput").ap()
    bq = nc.dram_tensor("bq", [CPC, 1], F32, kind="ExternalInput").ap()
    bk = nc.dram_tensor("bk", [CPC, 1], F32, kind="ExternalInput").ap()
    bv = nc.dram_tensor("bv", [CPC, 1], F32, kind="ExternalInput").ap()
    tri = nc.dram_tensor("tri", [128, 4, 512], CDT, kind="ExternalInput").ap()
    ident = nc.dram_tensor("ident", [128, 128], F32, kind="ExternalInput").ap()
    out = nc.dram_tensor("out", [S, D], F32, kind="ExternalOutput").ap()
    dbg_t = {}
    if dbg:
        for name, shape in (("d_QT", [128, S]), ("d_KT", [128, S]),
                            ("d_VP", [128, 2, 32, 65]), ("d_OT0", [64, S]),
                            ("d_OT1", [64, S]), ("d_pt", [128, 8, 1024]),
                            ("d_ot0", [65, 512]), ("d_ot1", [65, 512])):
            dbg_t[name] = nc.dram_tensor(name, shape, F32R,
                                         kind="ExternalOutput").ap()
        for name, shape in (("d_rec", [2, 512]), ("d_bc", [64, 512])):
            dbg_t[name] = nc.dram_tensor(name, shape, F32,
                                         kind="ExternalOutput").ap()

    with tile.TileContext(nc) as tc:
        _emit(nc, tc, xT, w3, wo, bq, bk, bv, tri, ident, out, dbg_t)
    nc.compile()
    return nc


def _emit(nc, tc, xT, w3, wo, bq, bk, bv, tri, ident, out, dbg_t=None):
    from contextlib import ExitStack
    ctx = ExitStack()
    with ctx:
        consts = ctx.enter_context(tc.tile_pool(name="consts", bufs=1))
        persist = ctx.enter_context(tc.tile_pool(name="persist", bufs=1))
        xt_pool = ctx.enter_context(tc.tile_pool(name="xt", bufs=24))
        vt_pool = ctx.enter_context(tc.tile_pool(name="vt", bufs=2))
        pt_pool = ctx.enter_context(tc.tile_pool(name="pt", bufs=6))
        rec_pool = ctx.enter_context(tc.tile_pool(name="rec", bufs=2))
        bc_pool = ctx.enter_context(tc.tile_pool(name="bc", bufs=2))
        ost_pool = ctx.enter_context(tc.tile_pool(name="ost", bufs=4))
        ps_mm = ctx.enter_context(tc.tile_pool(name="psmm", bufs=2, space="PSUM"))
        ps_sc = ctx.enter_context(tc.tile_pool(name="pssc", bufs=2, space="PSUM"))
        ps_ot = ctx.enter_context(tc.tile_pool(name="psot", bufs=2, space="PSUM"))

        # ---- constants (wq slice first so the first matmul starts early;
        # the rest after the first chunk's xt loads) ----------------------
        w_sb = consts.tile([128, 3, ND, 128], CDT)
        nc.sync.dma_start(out=w_sb[:, 0], in_=w3[:, 0])
        bq_sb = consts.tile([CPC, 1], F32)
        bk_sb = consts.tile([CPC, 1], F32)
        bv_sb = consts.tile([CPC, 1], F32)
        nc.sync.dma_start(out=bq_sb, in_=bq)
        nc.sync.dma_start(out=bk_sb, in_=bk)
        nc.sync.dma_start(out=bv_sb, in_=bv)
        id_sb = consts.tile([128, 128], F32)
        tri_sb = consts.tile([128, 4, 512], CDT)
        wo_sb = consts.tile([128, D], CDT)

        def emit_late_consts():
            nc.sync.dma_start(out=w_sb[:, 1], in_=w3[:, 1])
            nc.sync.dma_start(out=w_sb[:, 2], in_=w3[:, 2])
            nc.sync.dma_start(out=tri_sb, in_=tri)
            nc.sync.dma_start(out=id_sb, in_=ident)
            nc.sync.dma_start(out=wo_sb, in_=wo)

        # ---- persistent activations -----------------------------------
        QT = persist.tile([128, S], CDT)      # [c(2 heads x 64), s]
        KT = persist.tile([128, S], CDT)
        VP = persist.tile([128, 2, 32, 65], CDT)  # V natural + ones col, per (h, sk-tile)
        OT = persist.tile([128, S], CDT)      # rescaled attn out^T, both heads

        def emit_proj(c):
            # output projection for s-chunk c (deferred one chunk so the
            # rescale latency chain never stalls the PE queue)
            for t in range(4):
                ss = slice(128 * (4 * c + t), 128 * (4 * c + t + 1))
                for n in range(2):
                    nn = slice(512 * n, 512 * (n + 1))
                    pr_ps = ps_mm.tile([128, 512], F32, tag="mm")
                    nc.tensor.matmul(pr_ps, OT[:, ss], wo_sb[:, nn],
                                     start=True, stop=True)
                    o_sb = ost_pool.tile([128, 512], F32, tag="ost")
                    nc.vector.tensor_copy(o_sb, pr_ps)
                    nc.sync.dma_start(out=out[ss, nn], in_=o_sb)

        for c in range(NS):
            cc = slice(512 * c, 512 * (c + 1))

            # ---- phase A: QKV projections for s-chunk c ---------------
            xts = []
            for k in range(ND):
                xt = xt_pool.tile([128, 512], CDT, tag="xt")
                nc.sync.dma_start(out=xt, in_=xT[128 * k:128 * (k + 1), cc])
                xts.append(xt)
            if c == 0:
                emit_late_consts()
            vt_sb = vt_pool.tile([128, 512], F32, tag="vt")
            for i, (dst, b_sb) in enumerate(((QT[:, cc], bq_sb),
                                             (KT[:, cc], bk_sb),
                                             (vt_sb, bv_sb))):
                p_ps = ps_mm.tile([128, 512], F32, tag="mm")
                for k in range(ND):
                    nc.tensor.matmul(p_ps, w_sb[:, i, k], xts[k],
                                     start=(k == 0), stop=(k == ND - 1))
                nc.vector.tensor_scalar_add(dst, p_ps, b_sb)
            # V^T [c, s] -> V natural [s, c] per 128-block, + ones column
            for t in range(4):
                j = 4 * c + t
                tr_ps = ps_mm.tile([128, 128], F32, tag="mm")
                nc.tensor.transpose(tr_ps, vt_sb[:, 128 * t:128 * (t + 1)], id_sb)
                nc.vector.tensor_copy(VP[:, 0, j, 0:64], tr_ps[:, 0:64])
                nc.vector.tensor_copy(VP[:, 1, j, 0:64], tr_ps[:, 64:128])
                # ones column (Memset has no f32r encoding: use in*0+1)
                for h in (0, 1):
                    nc.vector.tensor_scalar(VP[:, h, j, 64:65],
                                            tri_sb[:, 0, 0:1],
                                            0.0, 1.0, mybir.AluOpType.mult,
                                            mybir.AluOpType.add)

            if c > 0:
                emit_proj(c - 1)

            # ---- phase B: attention for sq chunk c --------------------
            njt = 4 * (c + 1)  # causal sk tiles
            ot0 = ps_ot.tile([65, 512], F32, tag="ot")
            ot1 = ps_ot.tile([65, 512], F32, tag="ot")

            def emit_av(j, pt):
                st, sp = (j == 0), (j == njt - 1)
                nc.tensor.matmul(ot0, VP[:, 0, j], pt[:, 0:512], start=st, stop=sp)
                nc.tensor.matmul(ot1, VP[:, 1, j], pt[:, 512:1024], start=st, stop=sp)

            # software pipeline: attn@V for iteration j-1 is emitted after
            # scores+exp of iteration j, so the in-order PE queue never
            # waits on the ACT engine's exp.
            pending = None
            for j in range(njt):
                jj = slice(128 * j, 128 * (j + 1))
                sc_ps = ps_sc.tile([128, 1024], F32, tag="sc")
                nc.tensor.matmul(sc_ps[:, 0:512], KT[0:64, jj], QT[0:64, cc],
                                 start=True, stop=True)
                nc.tensor.matmul(sc_ps[:, 512:1024], KT[64:128, jj], QT[64:128, cc],
                                 start=True, stop=True)
                pt = pt_pool.tile([128, 1024], CDT, tag="pt")
                nc.scalar.activation(out=pt, in_=sc_ps, func=EXP)
                off = 128 * j - 512 * c
                if off >= 0:  # diagonal tile: mask (zeros below 128j, tri on diag)
                    t = j - 4 * c
                    for half in (0, 512):
                        nc.vector.tensor_mul(pt[:, half:half + off + 128],
                                             pt[:, half:half + off + 128],
                                             tri_sb[:, t, 0:off + 128])
                if pending is not None:
                    emit_av(*pending)
                pending = (j, pt)
            emit_av(*pending)

            if dbg_t and c == 1:
                for name, ot in (("d_ot0", ot0), ("d_ot1", ot1)):
                    o_cp = ost_pool.tile([65, 512], F32R, tag="dbg")
                    nc.vector.tensor_copy(o_cp, ot)
                    nc.sync.dma_start(out=dbg_t[name], in_=o_cp)

            # softmax denominators (row 64) -> rescale
            for h, ot in ((0, ot0), (1, ot1)):
                den = rec_pool.tile([1, 512], F32, tag="den")
                nc.vector.tensor_copy(den, ot[64:65, :])
                rec = rec_pool.tile([1, 512], F32, tag="rec")
                nc.vector._custom_dve(RECIPROCAL_APPROX_FAST, out=rec,
                                      in0=den,
                                      s0=RECIP_APPROX_FAST_CONSTS["s0"],
                                      s1=RECIP_APPROX_FAST_CONSTS["s1"],
                                      imm2=RECIP_APPROX_FAST_CONSTS["imm2"])
                bc = bc_pool.tile([64, 512], F32, tag="bc")
                nc.gpsimd.partition_broadcast(bc, rec)
                if dbg_t and c == 1:
                    nc.sync.dma_start(out=dbg_t["d_rec"][h:h + 1], in_=rec)
                    if h == 0:
                        nc.sync.dma_start(out=dbg_t["d_bc"], in_=bc)
                nc.vector.tensor_mul(OT[64 * h:64 * (h + 1), cc], ot[0:64, :], bc)

        emit_proj(NS - 1)

        if dbg_t:
            nc.sync.dma_start(out=dbg_t["d_QT"], in_=QT)
            nc.sync.dma_start(out=dbg_t["d_KT"], in_=KT)
            nc.sync.dma_start(out=dbg_t["d_VP"], in_=VP)
            nc.sync.dma_start(out=dbg_t["d_OT0"], in_=OT[0:64])
            nc.sync.dma_start(out=dbg_t["d_OT1"], in_=OT[64:128])


_NC_CACHE = {}


def _get_nc():
    if "nc" not in _NC_CACHE:
        _NC_CACHE["nc"] = _build_nc()
    return _NC_CACHE["nc"]


def make_in_maps(x, Wq, bq, Wk, bk, Wv, bv, Wo, bo):
    import ml_dtypes
    cdt = ml_dtypes.bfloat16 if USE_BF16 else np.float32
    x = np.asarray(x, np.float32).reshape(S, D)
    xT = np.ascontiguousarray(x.T).astype(cdt)
    scale = 1.0 / np.sqrt(HD)
    # tri[:, t, :]: zeros on cols < 128t, upper-tri on cols [128t, 128t+128),
    # ones beyond (multiplied region is cols [0, 128t+128) of a 512 chunk)
    tri = np.ones((128, 4, 512), np.float32)
    triu = np.triu(np.ones((128, 128), np.float32))
    for t in range(4):
        tri[:, t, :128 * t] = 0.0
        tri[:, t, 128 * t:128 * (t + 1)] = triu
    ident = np.eye(128, dtype=np.float32)
    in_maps = []
    for c in range(NCORES):
        cs = slice(CPC * c, CPC * (c + 1))
        # w3[p, proj, k, c2] = W[128k+p, c2] for the three projections
        w3 = np.stack([np.asarray(Wq)[:, cs] * scale,
                       np.asarray(Wk)[:, cs],
                       np.asarray(Wv)[:, cs]], axis=1)  # [D, 3, 128]
        w3 = np.ascontiguousarray(
            w3.reshape(ND, 128, 3, CPC).transpose(1, 2, 0, 3)).astype(cdt)
        in_maps.append({
            "xT": xT,
            "w3": w3,
            "wo": np.ascontiguousarray(np.asarray(Wo)[cs, :]).astype(cdt),
            "bq": np.ascontiguousarray(np.asarray(bq)[cs] * scale).reshape(CPC, 1),
            "bk": np.ascontiguousarray(np.asarray(bk)[cs]).reshape(CPC, 1),
            "bv": np.ascontiguousarray(np.asarray(bv)[cs]).reshape(CPC, 1),
            "tri": tri.astype(cdt),
            "ident": ident,
        })
    return in_maps


def kernel(x, Wq, bq, Wk, bk, Wv, bv, Wo, bo, _run_kwargs=None):
    nc = _get_nc()
    in_maps = make_in_maps(x, Wq, bq, Wk, bk, Wv, bv, Wo, bo)
    res = run_bass_kernel_spmd(nc, in_maps, list(range(NCORES)),
                               **(_run_kwargs or {}))
    acc = np.zeros((S, D), np.float64)
    for c in range(NCORES):
        acc += res.results[c]["out"]
    full = (acc + np.asarray(bo, np.float64)).astype(np.float32)
    if _run_kwargs is not None:
        _NC_CACHE["last_results"] = res
    return full.reshape(1, S, D)


# revision 51
# speedup vs baseline: 1.0278x; 1.0118x over previous
"""Causal multi-head attention (B=1, S=4096, D=1024, H=16, HD=64) on 8 TRN2
NeuronCores.

Sharding: tensor-parallel over heads. Core c owns heads [2c, 2c+1]: Wq/Wk/Wv
column slices (128 cols) and Wo row slice (128 rows). Each core computes its
partial output projection over the full sequence; the host sums the 8 partials
and adds bo (the row-parallel all-reduce done at gather time).

Device kernel layout (per core; matmul operands in bf16, fp32 PSUM accum,
flip USE_BF16 off for float32r if tighter accuracy is ever needed):
  - x is fed pre-transposed (xT [D, S]) so the QKV projections need no
    on-chip transpose: Q^T/K^T/V^T [c=128(2 heads x 64), s] = W.T @ xT.
  - scores are computed TRANSPOSED: scT[sk, sq] = K_h @ Q_h^T per 128-row
    sk-tile (the two heads' K=64 matmuls run concurrently via PE row
    tiling), which makes exp(scores) directly usable as the stationary
    operand of attn@V -- no P transposes. Softmax denominators come for
    free from a ones-column appended to V (row 64 of the attn@V
    accumulator); the 1/sqrt(HD) scale is folded into Wq/bq on the host;
    max-subtraction is skipped (scores ~ N(0,1), exp is safe in fp32).
  - V^T -> V via PE transposes of [128,128] blocks.
  - causal masking: one DVE multiply per diagonal tile against a
    host-provided mask that is zero below the 128-aligned diagonal band
    and upper-triangular on it.
  - rescale by the denominator reciprocal (fast DVE recip + gpsimd
    partition-broadcast), applied to attn-out^T before the output
    projection (the divide commutes through the linear projection).
  - the attn@V matmuls are software-pipelined one sk-tile behind the
    scores/exp so the in-order PE queue never waits on the ACT engine,
    and each chunk's output projection is deferred one chunk so the
    rescale latency chain is hidden behind the next chunk's QKV matmuls.
"""
import sys

sys.path.insert(0, "/opt/trn_rl_repo")

import numpy as np

import concourse.bacc as bacc
import concourse.mybir as mybir
import concourse.tile as tile
from concourse.bass_utils import run_bass_kernel_spmd
from concourse.dve_ops import RECIPROCAL_APPROX_FAST, RECIP_APPROX_FAST_CONSTS

F32 = mybir.dt.float32
F32R = mybir.dt.float32r
BF16 = mybir.dt.bfloat16
USE_BF16 = True
CDT = BF16 if USE_BF16 else F32R  # matmul-operand compute dtype
EXP = mybir.ActivationFunctionType.Exp

S, D, H, HD = 4096, 1024, 16, 64
NCORES = 8
CPC = D // NCORES  # 128 head-dim columns per core (2 heads)
NS = S // 512      # 8 chunks of 512 along the sequence
ND = D // 128      # 8 contraction chunks for the projections


def _build_nc(dbg=False):
    nc = bacc.Bacc("TRN2", target_bir_lowering=False, debug=False,
                   num_devices=NCORES)
    xT = nc.dram_tensor("xT", [D, S], CDT, kind="ExternalInput").ap()
    w3 = nc.dram_tensor("w3", [128, 3, ND, 128], CDT, kind="ExternalInput").ap()
    wo = nc.dram_tensor("wo", [128, D], CDT, kind="ExternalInput").ap()
    bq = nc.dram_tensor("bq", [CPC, 1], F32, kind="ExternalInput").ap()
    bk = nc.dram_tensor("bk", [CPC, 1], F32, kind="ExternalInput").ap()
    bv = nc.dram_tensor("bv", [CPC, 1], F32, kind="ExternalInput").ap()
    tri = nc.dram_tensor("tri", [128, 4, 512], CDT, kind="ExternalInput").ap()
    ident = nc.dram_tensor("ident", [128, 128], F32, kind="ExternalInput").ap()
    out = nc.dram_tensor("out", [S, D], F32, kind="ExternalOutput").ap()
    dbg_t = {}
    if dbg:
        for name, shape in (("d_QT", [128, S]), ("d_KT", [128, S]),
                            ("d_VP", [128, 2, 32, 65]), ("d_OT0", [64, S]),
                            ("d_OT1", [64, S]), ("d_pt", [128, 8, 1024]),
                            ("d_ot0", [65, 512]), ("d_ot1", [65, 512])):
            dbg_t[name] = nc.dram_tensor(name, shape, F32R,
                                         kind="ExternalOutput").ap()
        for name, shape in (("d_rec", [2, 512]), ("d_bc", [64, 512])):
            dbg_t[name] = nc.dram_tensor(name, shape, F32,
                                         kind="ExternalOutput").ap()

    with tile.TileContext(nc) as tc:
        _emit(nc, tc, xT, w3, wo, bq, bk, bv, tri, ident, out, dbg_t)
    nc.compile()
    return nc


def _emit(nc, tc, xT, w3, wo, bq, bk, bv, tri, ident, out, dbg_t=None):
    from contextlib import ExitStack
    ctx = ExitStack()
    with ctx:
        consts = ctx.enter_context(tc.tile_pool(name="consts", bufs=1))
        persist = ctx.enter_context(tc.tile_pool(name="persist", bufs=1))
        xt_pool = ctx.enter_context(tc.tile_pool(name="xt", bufs=24))
        vt_pool = ctx.enter_context(tc.tile_pool(name="vt", bufs=2))
        pt_pool = ctx.enter_context(tc.tile_pool(name="pt", bufs=6))
        rec_pool = ctx.enter_context(tc.tile_pool(name="rec", bufs=2))
        bc_pool = ctx.enter_context(tc.tile_pool(name="bc", bufs=2))
        ost_pool = ctx.enter_context(tc.tile_pool(name="ost", bufs=4))
        ps_mm = ctx.enter_context(tc.tile_pool(name="psmm", bufs=2, space="PSUM"))
        ps_sc = ctx.enter_context(tc.tile_pool(name="pssc", bufs=2, space="PSUM"))
        ps_ot = ctx.enter_context(tc.tile_pool(name="psot", bufs=2, space="PSUM"))

        # ---- constants (wq slice first so the first matmul starts early;
        # the rest after the first chunk's xt loads) ----------------------
        w_sb = consts.tile([128, 3, ND, 128], CDT)
        nc.sync.dma_start(out=w_sb[:, 0], in_=w3[:, 0])
        bq_sb = consts.tile([CPC, 1], F32)
        bk_sb = consts.tile([CPC, 1], F32)
        bv_sb = consts.tile([CPC, 1], F32)
        nc.sync.dma_start(out=bq_sb, in_=bq)
        nc.sync.dma_start(out=bk_sb, in_=bk)
        nc.sync.dma_start(out=bv_sb, in_=bv)
        id_sb = consts.tile([128, 128], F32)
        tri_sb = consts.tile([128, 4, 512], CDT)
        wo_sb = consts.tile([128, D], CDT)

        def emit_late_consts():
            nc.sync.dma_start(out=w_sb[:, 1], in_=w3[:, 1])
            nc.sync.dma_start(out=w_sb[:, 2], in_=w3[:, 2])
            nc.sync.dma_start(out=tri_sb, in_=tri)
            nc.sync.dma_start(out=id_sb, in_=ident)
            nc.sync.dma_start(out=wo_sb, in_=wo)

        # ---- persistent activations -----------------------------------
        QT = persist.tile([128, S], CDT)      # [c(2 heads x 64), s]
        KT = persist.tile([128, S], CDT)
        VP = persist.tile([128, 2, 32, 65], CDT)  # V natural + ones col, per (h, sk-tile)
        OT = persist.tile([128, S], CDT)      # rescaled attn out^T, both heads

        def emit_proj(c):
            # output projection for s-chunk c (deferred one chunk so the
            # rescale latency chain never stalls the PE queue)
            for t in range(4):
                ss = slice(128 * (4 * c + t), 128 * (4 * c + t + 1))
                for n in range(2):
                    nn = slice(512 * n, 512 * (n + 1))
                    pr_ps = ps_mm.tile([128, 512], F32, tag="mm")
                    nc.tensor.matmul(pr_ps, OT[:, ss], wo_sb[:, nn],
                                     start=True, stop=True)
                    o_sb = ost_pool.tile([128, 512], F32, tag="ost")
                    nc.vector.tensor_copy(o_sb, pr_ps)
                    nc.sync.dma_start(out=out[ss, nn], in_=o_sb)

        for c in range(NS):
            cc = slice(512 * c, 512 * (c + 1))

            # ---- phase A: QKV projections for s-chunk c ---------------
            xts = []
            for k in range(ND):
                xt = xt_pool.tile([128, 512], CDT, tag="xt")
                nc.sync.dma_start(out=xt, in_=xT[128 * k:128 * (k + 1), cc])
                xts.append(xt)
            if c == 0:
                emit_late_consts()
            vt_sb = vt_pool.tile([128, 512], F32, tag="vt")
            for i, (dst, b_sb) in enumerate(((QT[:, cc], bq_sb),
                                             (KT[:, cc], bk_sb),
                                             (vt_sb, bv_sb))):
                p_ps = ps_mm.tile([128, 512], F32, tag="mm")
                for k in range(ND):
                    nc.tensor.matmul(p_ps, w_sb[:, i, k], xts[k],
                                     start=(k == 0), stop=(k == ND - 1))
                nc.vector.tensor_scalar_add(dst, p_ps, b_sb)
            # V^T [c, s] -> V natural [s, c] per 128-block, + ones column
            for t in range(4):
                j = 4 * c + t
                tr_ps = ps_mm.tile([128, 128], F32, tag="mm")
                nc.tensor.transpose(tr_ps, vt_sb[:, 128 * t:128 * (t + 1)], id_sb)
                nc.vector.tensor_copy(VP[:, 0, j, 0:64], tr_ps[:, 0:64])
                nc.vector.tensor_copy(VP[:, 1, j, 0:64], tr_ps[:, 64:128])
                # ones column (Memset has no f32r encoding: use in*0+1)
                for h in (0, 1):
                    nc.vector.tensor_scalar(VP[:, h, j, 64:65],
                                            tri_sb[:, 0, 0:1],
                                            0.0, 1.0, mybir.AluOpType.mult,
                                            mybir.AluOpType.add)

            if c > 0:
                emit_proj(c - 1)

            # ---- phase B: attention for sq chunk c --------------------
            njt = 4 * (c + 1)  # causal sk tiles
            ot0 = ps_ot.tile([65, 512], F32, tag="ot")
            ot1 = ps_ot.tile([65, 512], F32, tag="ot")

            def emit_av(j, pt):
                st, sp = (j == 0), (j == njt - 1)
                nc.tensor.matmul(ot0, VP[:, 0, j], pt[:, 0:512], start=st, stop=sp)
                nc.tensor.matmul(ot1, VP[:, 1, j], pt[:, 512:1024], start=st, stop=sp)

            # software pipeline: attn@V for iteration j-1 is emitted after
            # scores+exp of iteration j, so the in-order PE queue never
            # waits on the ACT engine's exp.
            pending = []
            for j in range(njt):
                jj = slice(128 * j, 128 * (j + 1))
                sc_ps = ps_sc.tile([128, 1024], F32, tag="sc")
                nc.tensor.matmul(sc_ps[:, 0:512], KT[0:64, jj], QT[0:64, cc],
                                 start=True, stop=True)
                nc.tensor.matmul(sc_ps[:, 512:1024], KT[64:128, jj], QT[64:128, cc],
                                 start=True, stop=True)
                pt = pt_pool.tile([128, 1024], CDT, tag="pt")
                nc.scalar.activation(out=pt, in_=sc_ps, func=EXP)
                off = 128 * j - 512 * c
                if off >= 0:  # diagonal tile: mask (zeros below 128j, tri on diag)
                    t = j - 4 * c
                    for half in (0, 512):
                        nc.vector.tensor_mul(pt[:, half:half + off + 128],
                                             pt[:, half:half + off + 128],
                                             tri_sb[:, t, 0:off + 128])
                pending.append((j, pt))
                if len(pending) > 2:
                    emit_av(*pending.pop(0))
            for p in pending:
                emit_av(*p)

            if dbg_t and c == 1:
                for name, ot in (("d_ot0", ot0), ("d_ot1", ot1)):
                    o_cp = ost_pool.tile([65, 512], F32R, tag="dbg")
                    nc.vector.tensor_copy(o_cp, ot)
                    nc.sync.dma_start(out=dbg_t[name], in_=o_cp)

            # softmax denominators (row 64) -> rescale
            for h, ot in ((0, ot0), (1, ot1)):
                den = rec_pool.tile([1, 512], F32, tag="den")
                nc.vector.tensor_copy(den, ot[64:65, :])
                rec = rec_pool.tile([1, 512], F32, tag="rec")
                nc.vector._custom_dve(RECIPROCAL_APPROX_FAST, out=rec,
                                      in0=den,
                                      s0=RECIP_APPROX_FAST_CONSTS["s0"],
                                      s1=RECIP_APPROX_FAST_CONSTS["s1"],
                                      imm2=RECIP_APPROX_FAST_CONSTS["imm2"])
                bc = bc_pool.tile([64, 512], F32, tag="bc")
                nc.gpsimd.partition_broadcast(bc, rec)
                if dbg_t and c == 1:
                    nc.sync.dma_start(out=dbg_t["d_rec"][h:h + 1], in_=rec)
                    if h == 0:
                        nc.sync.dma_start(out=dbg_t["d_bc"], in_=bc)
                nc.vector.tensor_mul(OT[64 * h:64 * (h + 1), cc], ot[0:64, :], bc)

        emit_proj(NS - 1)

        if dbg_t:
            nc.sync.dma_start(out=dbg_t["d_QT"], in_=QT)
            nc.sync.dma_start(out=dbg_t["d_KT"], in_=KT)
            nc.sync.dma_start(out=dbg_t["d_VP"], in_=VP)
            nc.sync.dma_start(out=dbg_t["d_OT0"], in_=OT[0:64])
            nc.sync.dma_start(out=dbg_t["d_OT1"], in_=OT[64:128])


_NC_CACHE = {}


def _get_nc():
    if "nc" not in _NC_CACHE:
        _NC_CACHE["nc"] = _build_nc()
    return _NC_CACHE["nc"]


def make_in_maps(x, Wq, bq, Wk, bk, Wv, bv, Wo, bo):
    import ml_dtypes
    cdt = ml_dtypes.bfloat16 if USE_BF16 else np.float32
    x = np.asarray(x, np.float32).reshape(S, D)
    xT = np.ascontiguousarray(x.T).astype(cdt)
    scale = 1.0 / np.sqrt(HD)
    # tri[:, t, :]: zeros on cols < 128t, upper-tri on cols [128t, 128t+128),
    # ones beyond (multiplied region is cols [0, 128t+128) of a 512 chunk)
    tri = np.ones((128, 4, 512), np.float32)
    triu = np.triu(np.ones((128, 128), np.float32))
    for t in range(4):
        tri[:, t, :128 * t] = 0.0
        tri[:, t, 128 * t:128 * (t + 1)] = triu
    ident = np.eye(128, dtype=np.float32)
    in_maps = []
    for c in range(NCORES):
        cs = slice(CPC * c, CPC * (c + 1))
        # w3[p, proj, k, c2] = W[128k+p, c2] for the three projections
        w3 = np.stack([np.asarray(Wq)[:, cs] * scale,
                       np.asarray(Wk)[:, cs],
                       np.asarray(Wv)[:, cs]], axis=1)  # [D, 3, 128]
        w3 = np.ascontiguousarray(
            w3.reshape(ND, 128, 3, CPC).transpose(1, 2, 0, 3)).astype(cdt)
        in_maps.append({
            "xT": xT,
            "w3": w3,
            "wo": np.ascontiguousarray(np.asarray(Wo)[cs, :]).astype(cdt),
            "bq": np.ascontiguousarray(np.asarray(bq)[cs] * scale).reshape(CPC, 1),
            "bk": np.ascontiguousarray(np.asarray(bk)[cs]).reshape(CPC, 1),
            "bv": np.ascontiguousarray(np.asarray(bv)[cs]).reshape(CPC, 1),
            "tri": tri.astype(cdt),
            "ident": ident,
        })
    return in_maps


def kernel(x, Wq, bq, Wk, bk, Wv, bv, Wo, bo, _run_kwargs=None):
    nc = _get_nc()
    in_maps = make_in_maps(x, Wq, bq, Wk, bk, Wv, bv, Wo, bo)
    res = run_bass_kernel_spmd(nc, in_maps, list(range(NCORES)),
                               **(_run_kwargs or {}))
    acc = np.zeros((S, D), np.float64)
    for c in range(NCORES):
        acc += res.results[c]["out"]
    full = (acc + np.asarray(bo, np.float64)).astype(np.float32)
    if _run_kwargs is not None:
        _NC_CACHE["last_results"] = res
    return full.reshape(1, S, D)


# revision 54
# speedup vs baseline: 1.0508x; 1.0223x over previous
"""Causal multi-head attention (B=1, S=4096, D=1024, H=16, HD=64) on 8 TRN2
NeuronCores.

Sharding: tensor-parallel over heads. Core c owns heads [2c, 2c+1]: Wq/Wk/Wv
column slices (128 cols) and Wo row slice (128 rows). Each core computes its
partial output projection over the full sequence; the host sums the 8 partials
and adds bo (the row-parallel all-reduce done at gather time).

Device kernel layout (per core; matmul operands in bf16, fp32 PSUM accum,
flip USE_BF16 off for float32r if tighter accuracy is ever needed):
  - x is fed pre-transposed (xT [D, S]) so the QKV projections need no
    on-chip transpose: Q^T/K^T/V^T [c=128(2 heads x 64), s] = W.T @ xT.
  - scores are computed TRANSPOSED: scT[sk, sq] = K_h @ Q_h^T per 128-row
    sk-tile (the two heads' K=64 matmuls run concurrently via PE row
    tiling), which makes exp(scores) directly usable as the stationary
    operand of attn@V -- no P transposes. Softmax denominators come for
    free from a ones-column appended to V (row 64 of the attn@V
    accumulator); the 1/sqrt(HD) scale is folded into Wq/bq on the host;
    max-subtraction is skipped (scores ~ N(0,1), exp is safe in fp32).
  - V^T -> V via PE transposes of [128,128] blocks.
  - causal masking: one DVE multiply per diagonal tile against a
    host-provided mask that is zero below the 128-aligned diagonal band
    and upper-triangular on it.
  - rescale by the denominator reciprocal (fast DVE recip + gpsimd
    partition-broadcast), applied to attn-out^T before the output
    projection (the divide commutes through the linear projection).
  - the attn@V matmuls are software-pipelined one sk-tile behind the
    scores/exp so the in-order PE queue never waits on the ACT engine,
    and each chunk's output projection is deferred one chunk so the
    rescale latency chain is hidden behind the next chunk's QKV matmuls.
"""
import sys

sys.path.insert(0, "/opt/trn_rl_repo")

import numpy as np

import concourse.bacc as bacc
import concourse.mybir as mybir
import concourse.tile as tile
from concourse.bass_utils import run_bass_kernel_spmd
from concourse.dve_ops import RECIPROCAL_APPROX_FAST, RECIP_APPROX_FAST_CONSTS

F32 = mybir.dt.float32
F32R = mybir.dt.float32r
BF16 = mybir.dt.bfloat16
USE_BF16 = True
CDT = BF16 if USE_BF16 else F32R  # matmul-operand compute dtype
EXP = mybir.ActivationFunctionType.Exp

S, D, H, HD = 4096, 1024, 16, 64
NCORES = 8
CPC = D // NCORES  # 128 head-dim columns per core (2 heads)
NS = S // 512      # 8 chunks of 512 along the sequence
ND = D // 128      # 8 contraction chunks for the projections


def _build_nc(dbg=False):
    nc = bacc.Bacc("TRN2", target_bir_lowering=False, debug=False,
                   num_devices=NCORES)
    xT = nc.dram_tensor("xT", [D, S], CDT, kind="ExternalInput").ap()
    w3 = nc.dram_tensor("w3", [128, 3, ND, 128], CDT, kind="ExternalInput").ap()
    wo = nc.dram_tensor("wo", [128, D], CDT, kind="ExternalInput").ap()
    bq = nc.dram_tensor("bq", [CPC, 1], F32, kind="ExternalInput").ap()
    bk = nc.dram_tensor("bk", [CPC, 1], F32, kind="ExternalInput").ap()
    bv = nc.dram_tensor("bv", [CPC, 1], F32, kind="ExternalInput").ap()
    tri = nc.dram_tensor("tri", [128, 4, 512], CDT, kind="ExternalInput").ap()
    ident = nc.dram_tensor("ident", [128, 128], F32, kind="ExternalInput").ap()
    out = nc.dram_tensor("out", [S, D], F32, kind="ExternalOutput").ap()
    dbg_t = {}
    if dbg:
        for name, shape in (("d_QT", [128, S]), ("d_KT", [128, S]),
                            ("d_VP", [128, 2, 32, 65]), ("d_OT0", [64, S]),
                            ("d_OT1", [64, S]), ("d_pt", [128, 8, 1024]),
                            ("d_ot0", [65, 512]), ("d_ot1", [65, 512])):
            dbg_t[name] = nc.dram_tensor(name, shape, F32R,
                                         kind="ExternalOutput").ap()
        for name, shape in (("d_rec", [2, 512]), ("d_bc", [64, 512])):
            dbg_t[name] = nc.dram_tensor(name, shape, F32,
                                         kind="ExternalOutput").ap()

    with tile.TileContext(nc) as tc:
        _emit(nc, tc, xT, w3, wo, bq, bk, bv, tri, ident, out, dbg_t)
    nc.compile()
    return nc


def _emit(nc, tc, xT, w3, wo, bq, bk, bv, tri, ident, out, dbg_t=None):
    from contextlib import ExitStack
    ctx = ExitStack()
    with ctx:
        consts = ctx.enter_context(tc.tile_pool(name="consts", bufs=1))
        persist = ctx.enter_context(tc.tile_pool(name="persist", bufs=1))
        xt_pool = ctx.enter_context(tc.tile_pool(name="xt", bufs=3))
        vt_pool = ctx.enter_context(tc.tile_pool(name="vt", bufs=2))
        pt_pool = ctx.enter_context(tc.tile_pool(name="pt", bufs=6))
        rec_pool = ctx.enter_context(tc.tile_pool(name="rec", bufs=2))
        bc_pool = ctx.enter_context(tc.tile_pool(name="bc", bufs=2))
        ost_pool = ctx.enter_context(tc.tile_pool(name="ost", bufs=4))
        ps_mm = ctx.enter_context(tc.tile_pool(name="psmm", bufs=2, space="PSUM"))
        ps_sc = ctx.enter_context(tc.tile_pool(name="pssc", bufs=2, space="PSUM"))
        ps_ot = ctx.enter_context(tc.tile_pool(name="psot", bufs=2, space="PSUM"))

        # ---- constants (wq slice first so the first matmul starts early;
        # the rest after the first chunk's xt loads) ----------------------
        w_sb = consts.tile([128, 3, ND, 128], CDT)
        nc.sync.dma_start(out=w_sb[:, 0], in_=w3[:, 0])
        bq_sb = consts.tile([CPC, 1], F32)
        bk_sb = consts.tile([CPC, 1], F32)
        bv_sb = consts.tile([CPC, 1], F32)
        nc.sync.dma_start(out=bq_sb, in_=bq)
        nc.sync.dma_start(out=bk_sb, in_=bk)
        nc.sync.dma_start(out=bv_sb, in_=bv)
        id_sb = consts.tile([128, 128], F32)
        tri_sb = consts.tile([128, 4, 512], CDT)
        wo_sb = consts.tile([128, D], CDT)

        def emit_late_consts():
            nc.sync.dma_start(out=w_sb[:, 1], in_=w3[:, 1])
            nc.sync.dma_start(out=w_sb[:, 2], in_=w3[:, 2])
            nc.sync.dma_start(out=tri_sb, in_=tri)
            nc.sync.dma_start(out=id_sb, in_=ident)
            nc.sync.dma_start(out=wo_sb, in_=wo)

        # ---- persistent activations -----------------------------------
        QT = persist.tile([128, S], CDT)      # [c(2 heads x 64), s]
        KT = persist.tile([128, S], CDT)
        VP = persist.tile([128, 2, 32, 65], CDT)  # V natural + ones col, per (h, sk-tile)
        OT = persist.tile([128, S], CDT)      # rescaled attn out^T, both heads

        def emit_proj(c):
            # output projection for s-chunk c (deferred one chunk so the
            # rescale latency chain never stalls the PE queue)
            for t in range(4):
                ss = slice(128 * (4 * c + t), 128 * (4 * c + t + 1))
                for n in range(2):
                    nn = slice(512 * n, 512 * (n + 1))
                    pr_ps = ps_mm.tile([128, 512], F32, tag="mm")
                    nc.tensor.matmul(pr_ps, OT[:, ss], wo_sb[:, nn],
                                     start=True, stop=True)
                    o_sb = ost_pool.tile([128, 512], F32, tag="ost")
                    nc.vector.tensor_copy(o_sb, pr_ps)
                    nc.sync.dma_start(out=out[ss, nn], in_=o_sb)

        for c in range(NS):
            cc = slice(512 * c, 512 * (c + 1))

            # ---- phase A: QKV projections for s-chunk c ---------------
            xt = xt_pool.tile([128, ND, 512], CDT, tag="xt")
            nc.sync.dma_start(
                out=xt, in_=xT.rearrange("(k p) s -> p k s", p=128)[:, :, cc])
            xts = [xt[:, k] for k in range(ND)]
            if c == 0:
                emit_late_consts()
            vt_sb = vt_pool.tile([128, 512], F32, tag="vt")
            for i, (dst, b_sb) in enumerate(((QT[:, cc], bq_sb),
                                             (KT[:, cc], bk_sb),
                                             (vt_sb, bv_sb))):
                p_ps = ps_mm.tile([128, 512], F32, tag="mm")
                for k in range(ND):
                    nc.tensor.matmul(p_ps, w_sb[:, i, k], xts[k],
                                     start=(k == 0), stop=(k == ND - 1))
                nc.vector.tensor_scalar_add(dst, p_ps, b_sb)
            # V^T [c, s] -> V natural [s, c] per 128-block, + ones column
            for t in range(4):
                j = 4 * c + t
                tr_ps = ps_mm.tile([128, 128], F32, tag="mm")
                nc.tensor.transpose(tr_ps, vt_sb[:, 128 * t:128 * (t + 1)], id_sb)
                nc.vector.tensor_copy(VP[:, 0, j, 0:64], tr_ps[:, 0:64])
                nc.vector.tensor_copy(VP[:, 1, j, 0:64], tr_ps[:, 64:128])
                # ones column (Memset has no f32r encoding: use in*0+1)
                for h in (0, 1):
                    nc.vector.tensor_scalar(VP[:, h, j, 64:65],
                                            tri_sb[:, 0, 0:1],
                                            0.0, 1.0, mybir.AluOpType.mult,
                                            mybir.AluOpType.add)

            if c > 0:
                emit_proj(c - 1)

            # ---- phase B: attention for sq chunk c --------------------
            njt = 4 * (c + 1)  # causal sk tiles
            ot0 = ps_ot.tile([65, 512], F32, tag="ot")
            ot1 = ps_ot.tile([65, 512], F32, tag="ot")

            def emit_av(j, pt):
                st, sp = (j == 0), (j == njt - 1)
                nc.tensor.matmul(ot0, VP[:, 0, j], pt[:, 0:512], start=st, stop=sp)
                nc.tensor.matmul(ot1, VP[:, 1, j], pt[:, 512:1024], start=st, stop=sp)

            # software pipeline: attn@V for iteration j-1 is emitted after
            # scores+exp of iteration j, so the in-order PE queue never
            # waits on the ACT engine's exp.
            pending = []
            for j in range(njt):
                jj = slice(128 * j, 128 * (j + 1))
                sc_ps = ps_sc.tile([128, 1024], F32, tag="sc")
                nc.tensor.matmul(sc_ps[:, 0:512], KT[0:64, jj], QT[0:64, cc],
                                 start=True, stop=True)
                nc.tensor.matmul(sc_ps[:, 512:1024], KT[64:128, jj], QT[64:128, cc],
                                 start=True, stop=True)
                pt = pt_pool.tile([128, 1024], CDT, tag="pt")
                nc.scalar.activation(out=pt, in_=sc_ps, func=EXP)
                off = 128 * j - 512 * c
                if off >= 0:  # diagonal tile: mask (zeros below 128j, tri on diag)
                    t = j - 4 * c
                    for half in (0, 512):
                        nc.vector.tensor_mul(pt[:, half:half + off + 128],
                                             pt[:, half:half + off + 128],
                                             tri_sb[:, t, 0:off + 128])
                pending.append((j, pt))
                if len(pending) > 2:
                    emit_av(*pending.pop(0))
            for p in pending:
                emit_av(*p)

            if dbg_t and c == 1:
                for name, ot in (("d_ot0", ot0), ("d_ot1", ot1)):
                    o_cp = ost_pool.tile([65, 512], F32R, tag="dbg")
                    nc.vector.tensor_copy(o_cp, ot)
                    nc.sync.dma_start(out=dbg_t[name], in_=o_cp)

            # softmax denominators (row 64) -> rescale; stage-interleaved so
            # the DVE and gpsimd legs of the two heads pipeline
            dens, recs, bcs = [], [], []
            for h, ot in ((0, ot0), (1, ot1)):
                den = rec_pool.tile([1, 512], F32, tag="den")
                nc.vector.tensor_copy(den, ot[64:65, :])
                dens.append(den)
            for h in (0, 1):
                rec = rec_pool.tile([1, 512], F32, tag="rec")
                nc.vector._custom_dve(RECIPROCAL_APPROX_FAST, out=rec,
                                      in0=dens[h],
                                      s0=RECIP_APPROX_FAST_CONSTS["s0"],
                                      s1=RECIP_APPROX_FAST_CONSTS["s1"],
                                      imm2=RECIP_APPROX_FAST_CONSTS["imm2"])
                recs.append(rec)
                bc = bc_pool.tile([64, 512], F32, tag="bc")
                nc.gpsimd.partition_broadcast(bc, rec)
                bcs.append(bc)
            for h, ot in ((0, ot0), (1, ot1)):
                nc.vector.tensor_mul(OT[64 * h:64 * (h + 1), cc], ot[0:64, :],
                                     bcs[h])

        emit_proj(NS - 1)

        if dbg_t:
            nc.sync.dma_start(out=dbg_t["d_QT"], in_=QT)
            nc.sync.dma_start(out=dbg_t["d_KT"], in_=KT)
            nc.sync.dma_start(out=dbg_t["d_VP"], in_=VP)
            nc.sync.dma_start(out=dbg_t["d_OT0"], in_=OT[0:64])
            nc.sync.dma_start(out=dbg_t["d_OT1"], in_=OT[64:128])


_NC_CACHE = {}


def _get_nc():
    if "nc" not in _NC_CACHE:
        _NC_CACHE["nc"] = _build_nc()
    return _NC_CACHE["nc"]


def make_in_maps(x, Wq, bq, Wk, bk, Wv, bv, Wo, bo):
    import ml_dtypes
    cdt = ml_dtypes.bfloat16 if USE_BF16 else np.float32
    x = np.asarray(x, np.float32).reshape(S, D)
    xT = np.ascontiguousarray(x.T).astype(cdt)
    scale = 1.0 / np.sqrt(HD)
    # tri[:, t, :]: zeros on cols < 128t, upper-tri on cols [128t, 128t+128),
    # ones beyond (multiplied region is cols [0, 128t+128) of a 512 chunk)
    tri = np.ones((128, 4, 512), np.float32)
    triu = np.triu(np.ones((128, 128), np.float32))
    for t in range(4):
        tri[:, t, :128 * t] = 0.0
        tri[:, t, 128 * t:128 * (t + 1)] = triu
    ident = np.eye(128, dtype=np.float32)
    in_maps = []
    for c in range(NCORES):
        cs = slice(CPC * c, CPC * (c + 1))
        # w3[p, proj, k, c2] = W[128k+p, c2] for the three projections
        w3 = np.stack([np.asarray(Wq)[:, cs] * scale,
                       np.asarray(Wk)[:, cs],
                       np.asarray(Wv)[:, cs]], axis=1)  # [D, 3, 128]
        w3 = np.ascontiguousarray(
            w3.reshape(ND, 128, 3, CPC).transpose(1, 2, 0, 3)).astype(cdt)
        in_maps.append({
            "xT": xT,
            "w3": w3,
            "wo": np.ascontiguousarray(np.asarray(Wo)[cs, :]).astype(cdt),
            "bq": np.ascontiguousarray(np.asarray(bq)[cs] * scale).reshape(CPC, 1),
            "bk": np.ascontiguousarray(np.asarray(bk)[cs]).reshape(CPC, 1),
            "bv": np.ascontiguousarray(np.asarray(bv)[cs]).reshape(CPC, 1),
            "tri": tri.astype(cdt),
            "ident": ident,
        })
    return in_maps


def kernel(x, Wq, bq, Wk, bk, Wv, bv, Wo, bo, _run_kwargs=None):
    nc = _get_nc()
    in_maps = make_in_maps(x, Wq, bq, Wk, bk, Wv, bv, Wo, bo)
    res = run_bass_kernel_spmd(nc, in_maps, list(range(NCORES)),
                               **(_run_kwargs or {}))
    acc = np.zeros((S, D), np.float64)
    for c in range(NCORES):
        acc += res.results[c]["out"]
    full = (acc + np.asarray(bo, np.float64)).astype(np.float32)
    if _run_kwargs is not None:
        _NC_CACHE["last_results"] = res
    return full.reshape(1, S, D)


# revision 55
# speedup vs baseline: 1.0625x; 1.0112x over previous
"""Causal multi-head attention (B=1, S=4096, D=1024, H=16, HD=64) on 8 TRN2
NeuronCores.

Sharding: tensor-parallel over heads. Core c owns heads [2c, 2c+1]: Wq/Wk/Wv
column slices (128 cols) and Wo row slice (128 rows). Each core computes its
partial output projection over the full sequence; the host sums the 8 partials
and adds bo (the row-parallel all-reduce done at gather time).

Device kernel layout (per core; matmul operands in bf16, fp32 PSUM accum,
flip USE_BF16 off for float32r if tighter accuracy is ever needed):
  - x is fed pre-transposed (xT [D, S]) so the QKV projections need no
    on-chip transpose: Q^T/K^T/V^T [c=128(2 heads x 64), s] = W.T @ xT.
  - scores are computed TRANSPOSED: scT[sk, sq] = K_h @ Q_h^T per 128-row
    sk-tile (the two heads' K=64 matmuls run concurrently via PE row
    tiling), which makes exp(scores) directly usable as the stationary
    operand of attn@V -- no P transposes. Softmax denominators come for
    free from a ones-column appended to V (row 64 of the attn@V
    accumulator); the 1/sqrt(HD) scale is folded into Wq/bq on the host;
    max-subtraction is skipped (scores ~ N(0,1), exp is safe in fp32).
  - V^T -> V via PE transposes of [128,128] blocks.
  - causal masking: one DVE multiply per diagonal tile against a
    host-provided mask that is zero below the 128-aligned diagonal band
    and upper-triangular on it.
  - rescale by the denominator reciprocal (fast DVE recip + gpsimd
    partition-broadcast), applied to attn-out^T before the output
    projection (the divide commutes through the linear projection).
  - the attn@V matmuls are software-pipelined one sk-tile behind the
    scores/exp so the in-order PE queue never waits on the ACT engine,
    and each chunk's output projection is deferred one chunk so the
    rescale latency chain is hidden behind the next chunk's QKV matmuls.
"""
import sys

sys.path.insert(0, "/opt/trn_rl_repo")

import numpy as np

import concourse.bacc as bacc
import concourse.mybir as mybir
import concourse.tile as tile
from concourse.bass_utils import run_bass_kernel_spmd
from concourse.dve_ops import RECIPROCAL_APPROX_FAST, RECIP_APPROX_FAST_CONSTS

F32 = mybir.dt.float32
F32R = mybir.dt.float32r
BF16 = mybir.dt.bfloat16
USE_BF16 = True
CDT = BF16 if USE_BF16 else F32R  # matmul-operand compute dtype
EXP = mybir.ActivationFunctionType.Exp

S, D, H, HD = 4096, 1024, 16, 64
NCORES = 8
CPC = D // NCORES  # 128 head-dim columns per core (2 heads)
NS = S // 512      # 8 chunks of 512 along the sequence
ND = D // 128      # 8 contraction chunks for the projections


def _build_nc(dbg=False):
    nc = bacc.Bacc("TRN2", target_bir_lowering=False, debug=False,
                   num_devices=NCORES)
    xT = nc.dram_tensor("xT", [D, S], CDT, kind="ExternalInput").ap()
    w3 = nc.dram_tensor("w3", [128, 3, ND, 128], CDT, kind="ExternalInput").ap()
    wo = nc.dram_tensor("wo", [128, D], CDT, kind="ExternalInput").ap()
    bq = nc.dram_tensor("bq", [CPC, 1], F32, kind="ExternalInput").ap()
    bk = nc.dram_tensor("bk", [CPC, 1], F32, kind="ExternalInput").ap()
    bv = nc.dram_tensor("bv", [CPC, 1], F32, kind="ExternalInput").ap()
    tri = nc.dram_tensor("tri", [128, 4, 512], CDT, kind="ExternalInput").ap()
    ident = nc.dram_tensor("ident", [128, 128], F32, kind="ExternalInput").ap()
    out = nc.dram_tensor("out", [S, D], F32, kind="ExternalOutput").ap()
    dbg_t = {}
    if dbg:
        for name, shape in (("d_QT", [128, S]), ("d_KT", [128, S]),
                            ("d_VP", [128, 2, 32, 65]), ("d_OT0", [64, S]),
                            ("d_OT1", [64, S]), ("d_pt", [128, 8, 1024]),
                            ("d_ot0", [65, 512]), ("d_ot1", [65, 512])):
            dbg_t[name] = nc.dram_tensor(name, shape, F32R,
                                         kind="ExternalOutput").ap()
        for name, shape in (("d_rec", [2, 512]), ("d_bc", [64, 512])):
            dbg_t[name] = nc.dram_tensor(name, shape, F32,
                                         kind="ExternalOutput").ap()

    with tile.TileContext(nc) as tc:
        _emit(nc, tc, xT, w3, wo, bq, bk, bv, tri, ident, out, dbg_t)
    nc.compile()
    return nc


def _emit(nc, tc, xT, w3, wo, bq, bk, bv, tri, ident, out, dbg_t=None):
    from contextlib import ExitStack
    ctx = ExitStack()
    with ctx:
        consts = ctx.enter_context(tc.tile_pool(name="consts", bufs=1))
        persist = ctx.enter_context(tc.tile_pool(name="persist", bufs=1))
        xt_pool = ctx.enter_context(tc.tile_pool(name="xt", bufs=3))
        vt_pool = ctx.enter_context(tc.tile_pool(name="vt", bufs=2))
        pt_pool = ctx.enter_context(tc.tile_pool(name="pt", bufs=6))
        rec_pool = ctx.enter_context(tc.tile_pool(name="rec", bufs=2))
        bc_pool = ctx.enter_context(tc.tile_pool(name="bc", bufs=2))
        ost_pool = ctx.enter_context(tc.tile_pool(name="ost", bufs=4))
        ps_mm = ctx.enter_context(tc.tile_pool(name="psmm", bufs=2, space="PSUM"))
        ps_sc = ctx.enter_context(tc.tile_pool(name="pssc", bufs=2, space="PSUM"))
        ps_ot = ctx.enter_context(tc.tile_pool(name="psot", bufs=2, space="PSUM"))

        # ---- constants (wq slice first so the first matmul starts early;
        # the rest after the first chunk's xt loads) ----------------------
        w_sb = consts.tile([128, 3, ND, 128], CDT)
        nc.sync.dma_start(out=w_sb[:, 0], in_=w3[:, 0])
        bq_sb = consts.tile([CPC, 1], F32)
        bk_sb = consts.tile([CPC, 1], F32)
        bv_sb = consts.tile([CPC, 1], F32)
        nc.sync.dma_start(out=bq_sb, in_=bq)
        nc.sync.dma_start(out=bk_sb, in_=bk)
        nc.sync.dma_start(out=bv_sb, in_=bv)
        id_sb = consts.tile([128, 128], F32)
        tri_sb = consts.tile([128, 4, 512], CDT)
        wo_sb = consts.tile([128, D], CDT)

        def emit_late_consts():
            nc.sync.dma_start(out=w_sb[:, 1], in_=w3[:, 1])
            nc.sync.dma_start(out=w_sb[:, 2], in_=w3[:, 2])
            nc.sync.dma_start(out=tri_sb, in_=tri)
            nc.sync.dma_start(out=id_sb, in_=ident)
            nc.sync.dma_start(out=wo_sb, in_=wo)

        # ---- persistent activations -----------------------------------
        QT = persist.tile([128, S], CDT)      # [c(2 heads x 64), s]
        KT = persist.tile([128, S], CDT)
        VP = persist.tile([128, 2, 32, 65], CDT)  # V natural + ones col, per (h, sk-tile)
        OT = persist.tile([128, S], CDT)      # rescaled attn out^T, both heads

        def emit_proj(c):
            # output projection for s-chunk c (deferred one chunk so the
            # rescale latency chain never stalls the PE queue)
            for t in range(4):
                ss = slice(128 * (4 * c + t), 128 * (4 * c + t + 1))
                for n in range(2):
                    nn = slice(512 * n, 512 * (n + 1))
                    pr_ps = ps_mm.tile([128, 512], F32, tag="mm")
                    nc.tensor.matmul(pr_ps, OT[:, ss], wo_sb[:, nn],
                                     start=True, stop=True)
                    o_sb = ost_pool.tile([128, 512], F32, tag="ost")
                    nc.vector.tensor_copy(o_sb, pr_ps)
                    nc.sync.dma_start(out=out[ss, nn], in_=o_sb)

        for c in range(NS):
            cc = slice(512 * c, 512 * (c + 1))

            # ---- phase A: QKV projections for s-chunk c ---------------
            xt = xt_pool.tile([128, ND, 512], CDT, tag="xt")
            xT_k = xT.rearrange("(k p) s -> p k s", p=128)
            if c == 0:
                # split the first load so the first matmul starts early
                nc.sync.dma_start(out=xt[:, 0:2], in_=xT_k[:, 0:2, cc])
                nc.sync.dma_start(out=xt[:, 2:ND], in_=xT_k[:, 2:ND, cc])
                emit_late_consts()
            else:
                nc.sync.dma_start(out=xt, in_=xT_k[:, :, cc])
            xts = [xt[:, k] for k in range(ND)]
            vt_sb = vt_pool.tile([128, 512], F32, tag="vt")
            for i, (dst, b_sb) in enumerate(((QT[:, cc], bq_sb),
                                             (KT[:, cc], bk_sb),
                                             (vt_sb, bv_sb))):
                p_ps = ps_mm.tile([128, 512], F32, tag="mm")
                for k in range(ND):
                    nc.tensor.matmul(p_ps, w_sb[:, i, k], xts[k],
                                     start=(k == 0), stop=(k == ND - 1))
                nc.vector.tensor_scalar_add(dst, p_ps, b_sb)
            # V^T [c, s] -> V natural [s, c] per 128-block, + ones column
            for t in range(4):
                j = 4 * c + t
                tr_ps = ps_mm.tile([128, 128], F32, tag="mm")
                nc.tensor.transpose(tr_ps, vt_sb[:, 128 * t:128 * (t + 1)], id_sb)
                nc.vector.tensor_copy(VP[:, 0, j, 0:64], tr_ps[:, 0:64])
                nc.vector.tensor_copy(VP[:, 1, j, 0:64], tr_ps[:, 64:128])
                # ones column (Memset has no f32r encoding: use in*0+1)
                for h in (0, 1):
                    nc.vector.tensor_scalar(VP[:, h, j, 64:65],
                                            tri_sb[:, 0, 0:1],
                                            0.0, 1.0, mybir.AluOpType.mult,
                                            mybir.AluOpType.add)

            if c > 0:
                emit_proj(c - 1)

            # ---- phase B: attention for sq chunk c --------------------
            njt = 4 * (c + 1)  # causal sk tiles
            ot0 = ps_ot.tile([65, 512], F32, tag="ot")
            ot1 = ps_ot.tile([65, 512], F32, tag="ot")

            def emit_av(j, pt):
                st, sp = (j == 0), (j == njt - 1)
                nc.tensor.matmul(ot0, VP[:, 0, j], pt[:, 0:512], start=st, stop=sp)
                nc.tensor.matmul(ot1, VP[:, 1, j], pt[:, 512:1024], start=st, stop=sp)

            # software pipeline: attn@V for iteration j-1 is emitted after
            # scores+exp of iteration j, so the in-order PE queue never
            # waits on the ACT engine's exp.
            pending = []
            for j in range(njt):
                jj = slice(128 * j, 128 * (j + 1))
                sc_ps = ps_sc.tile([128, 1024], F32, tag="sc")
                nc.tensor.matmul(sc_ps[:, 0:512], KT[0:64, jj], QT[0:64, cc],
                                 start=True, stop=True)
                nc.tensor.matmul(sc_ps[:, 512:1024], KT[64:128, jj], QT[64:128, cc],
                                 start=True, stop=True)
                pt = pt_pool.tile([128, 1024], CDT, tag="pt")
                nc.scalar.activation(out=pt, in_=sc_ps, func=EXP)
                off = 128 * j - 512 * c
                if off >= 0:  # diagonal tile: mask (zeros below 128j, tri on diag)
                    t = j - 4 * c
                    for half in (0, 512):
                        nc.vector.tensor_mul(pt[:, half:half + off + 128],
                                             pt[:, half:half + off + 128],
                                             tri_sb[:, t, 0:off + 128])
                pending.append((j, pt))
                if len(pending) > 2:
                    emit_av(*pending.pop(0))
            for p in pending:
                emit_av(*p)

            if dbg_t and c == 1:
                for name, ot in (("d_ot0", ot0), ("d_ot1", ot1)):
                    o_cp = ost_pool.tile([65, 512], F32R, tag="dbg")
                    nc.vector.tensor_copy(o_cp, ot)
                    nc.sync.dma_start(out=dbg_t[name], in_=o_cp)

            # softmax denominators (row 64) -> rescale; stage-interleaved so
            # the DVE and gpsimd legs of the two heads pipeline
            dens, recs, bcs = [], [], []
            for h, ot in ((0, ot0), (1, ot1)):
                den = rec_pool.tile([1, 512], F32, tag="den")
                nc.vector.tensor_copy(den, ot[64:65, :])
                dens.append(den)
            for h in (0, 1):
                rec = rec_pool.tile([1, 512], F32, tag="rec")
                nc.vector._custom_dve(RECIPROCAL_APPROX_FAST, out=rec,
                                      in0=dens[h],
                                      s0=RECIP_APPROX_FAST_CONSTS["s0"],
                                      s1=RECIP_APPROX_FAST_CONSTS["s1"],
                                      imm2=RECIP_APPROX_FAST_CONSTS["imm2"])
                recs.append(rec)
                bc = bc_pool.tile([64, 512], F32, tag="bc")
                nc.gpsimd.partition_broadcast(bc, rec)
                bcs.append(bc)
            for h, ot in ((0, ot0), (1, ot1)):
                nc.vector.tensor_mul(OT[64 * h:64 * (h + 1), cc], ot[0:64, :],
                                     bcs[h])

        emit_proj(NS - 1)

        if dbg_t:
            nc.sync.dma_start(out=dbg_t["d_QT"], in_=QT)
            nc.sync.dma_start(out=dbg_t["d_KT"], in_=KT)
            nc.sync.dma_start(out=dbg_t["d_VP"], in_=VP)
            nc.sync.dma_start(out=dbg_t["d_OT0"], in_=OT[0:64])
            nc.sync.dma_start(out=dbg_t["d_OT1"], in_=OT[64:128])


_NC_CACHE = {}


def _get_nc():
    if "nc" not in _NC_CACHE:
        _NC_CACHE["nc"] = _build_nc()
    return _NC_CACHE["nc"]


def make_in_maps(x, Wq, bq, Wk, bk, Wv, bv, Wo, bo):
    import ml_dtypes
    cdt = ml_dtypes.bfloat16 if USE_BF16 else np.float32
    x = np.asarray(x, np.float32).reshape(S, D)
    xT = np.ascontiguousarray(x.T).astype(cdt)
    scale = 1.0 / np.sqrt(HD)
    # tri[:, t, :]: zeros on cols < 128t, upper-tri on cols [128t, 128t+128),
    # ones beyond (multiplied region is cols [0, 128t+128) of a 512 chunk)
    tri = np.ones((128, 4, 512), np.float32)
    triu = np.triu(np.ones((128, 128), np.float32))
    for t in range(4):
        tri[:, t, :128 * t] = 0.0
        tri[:, t, 128 * t:128 * (t + 1)] = triu
    ident = np.eye(128, dtype=np.float32)
    in_maps = []
    for c in range(NCORES):
        cs = slice(CPC * c, CPC * (c + 1))
        # w3[p, proj, k, c2] = W[128k+p, c2] for the three projections
        w3 = np.stack([np.asarray(Wq)[:, cs] * scale,
                       np.asarray(Wk)[:, cs],
                       np.asarray(Wv)[:, cs]], axis=1)  # [D, 3, 128]
        w3 = np.ascontiguousarray(
            w3.reshape(ND, 128, 3, CPC).transpose(1, 2, 0, 3)).astype(cdt)
        in_maps.append({
            "xT": xT,
            "w3": w3,
            "wo": np.ascontiguousarray(np.asarray(Wo)[cs, :]).astype(cdt),
            "bq": np.ascontiguousarray(np.asarray(bq)[cs] * scale).reshape(CPC, 1),
            "bk": np.ascontiguousarray(np.asarray(bk)[cs]).reshape(CPC, 1),
            "bv": np.ascontiguousarray(np.asarray(bv)[cs]).reshape(CPC, 1),
            "tri": tri.astype(cdt),
            "ident": ident,
        })
    return in_maps


def kernel(x, Wq, bq, Wk, bk, Wv, bv, Wo, bo, _run_kwargs=None):
    nc = _get_nc()
    in_maps = make_in_maps(x, Wq, bq, Wk, bk, Wv, bv, Wo, bo)
    res = run_bass_kernel_spmd(nc, in_maps, list(range(NCORES)),
                               **(_run_kwargs or {}))
    acc = np.zeros((S, D), np.float64)
    for c in range(NCORES):
        acc += res.results[c]["out"]
    full = (acc + np.asarray(bo, np.float64)).astype(np.float32)
    if _run_kwargs is not None:
        _NC_CACHE["last_results"] = res
    return full.reshape(1, S, D)
